# revision 33
# baseline (speedup 1.0000x reference)
"""Llama attention (N=2, S=2048, H=2048, nh=16, dh=128) on 8 NeuronCores.

v5: restructured for dense PE occupancy (HW ~343us vs 416us baseline):
- softmax denominator via ones-matrix matmul (M=128) so the PSUM result
  is already broadcast across partitions (no broadcast-mm + copy chain).
- 1/sum computed as exp(-ln(sum)) on ScalarE (ln+exp share one act
  table) instead of the 3.3us VectorE reciprocal.
- software-pipelined emission: score mms of head h+1 / next q-chunk and
  the previous q-chunk's output-projection mms interleave with
  denominator+context mms so the PE queue never drains (keeps HAM at
  K=8/8 through the attention phase).
- attention(b0) runs between the projection halves; b1's xt chunks DMA
  during it; chunk 3's V matmuls become warmup filler for attention b0;
  q-chunks processed 1,0,2,3 so the thin qc0 runs filled; the final
  out-proj drain alternates onto the idle score PSUM banks; denominator
  matmuls trail their own head's score phase by one pair so both
  1/sum chains finish before the context phases end.
- kernel() reuses one cached jitted executable across calls.
"""

import math
from collections import deque
from functools import lru_cache

import numpy as np
import ml_dtypes

N_CORES = 8
N, S, H = 2, 2048, 2048
NH, DH = 16, 128
HPC = NH // N_CORES          # heads per core = 2
T = N * S                    # 4096 tokens
P = 128
KI = H // P                  # 16 contraction subtiles for projections
TCH = 512                    # projection token chunk
QCH = 512                    # attention q chunk
SB = S // P                  # 16 key blocks per batch
HALF = DH // 2


def _build_nc(repeat=1):
    import concourse.mybir as mybir
    import concourse.tile as tile
    from concourse import bacc

    fp32 = mybir.dt.float32
    bf16 = mybir.dt.bfloat16
    EXP = mybir.ActivationFunctionType.Exp
    LN = mybir.ActivationFunctionType.Ln
    COPY = mybir.ActivationFunctionType.Copy

    nc = bacc.Bacc("TRN2", target_bir_lowering=False, debug=False,
                   num_devices=N_CORES)
    # xtc[c, p, k, t] = X.T[(k*128+p), c*512 + t]   (chunk-major, contiguous)
    xtc = nc.dram_tensor("xtc", [T // TCH, P, KI, TCH], bf16,
                         kind="ExternalInput")
    wqt = nc.dram_tensor("wqt", [P, KI, HPC * DH], bf16, kind="ExternalInput")
    wkt = nc.dram_tensor("wkt", [P, KI, HPC * DH], bf16, kind="ExternalInput")
    wvt = nc.dram_tensor("wvt", [P, KI, HPC * DH], bf16, kind="ExternalInput")
    wot = nc.dram_tensor("wot", [P, HPC, H], bf16, kind="ExternalInput")
    cos2 = nc.dram_tensor("cos2", [P, S], fp32, kind="ExternalInput")
    sinp = nc.dram_tensor("sinp", [HALF, S], fp32, kind="ExternalInput")
    tril = nc.dram_tensor("tril", [P, P], bf16, kind="ExternalInput")
    out = nc.dram_tensor("out", [T, H], bf16, kind="ExternalOutput")

    inv_sqrt_dh = 1.0 / math.sqrt(DH)
    n_tch = T // TCH            # 8 projection chunks
    n_qch = S // QCH            # 4 attention q-chunks per (head, batch)

    from contextlib import ExitStack

    with tile.TileContext(nc) as tc, ExitStack() as es:
        consts = es.enter_context(tc.tile_pool(name="consts", bufs=1))
        wpool = es.enter_context(tc.tile_pool(name="wpool", bufs=1))
        xtp = es.enter_context(tc.tile_pool(name="xtp", bufs=2))
        qkv = es.enter_context(tc.tile_pool(name="qkv", bufs=1))
        wt_pool = es.enter_context(tc.tile_pool(name="wt", bufs=2))
        ctx_pool = es.enter_context(tc.tile_pool(name="ctxp", bufs=2))
        outp = es.enter_context(tc.tile_pool(name="outp", bufs=3))
        tmp = es.enter_context(tc.tile_pool(name="tmp", bufs=2))
        # PSUM: sc 2x2 + dn 1 + cx 1 + op 2 = 8 banks
        ps_sc = es.enter_context(tc.tile_pool(name="ps_sc", bufs=2,
                                              space="PSUM"))
        ps_dn = es.enter_context(tc.tile_pool(name="ps_dn", bufs=1,
                                              space="PSUM"))
        ps_cx = es.enter_context(tc.tile_pool(name="ps_cx", bufs=1,
                                              space="PSUM"))
        ps_op = es.enter_context(tc.tile_pool(name="ps_op", bufs=2,
                                              space="PSUM"))

        # ---- first xt chunk + first wq quarter DMAs before anything ----
        xt_tiles = {}
        xt_tiles[0] = xtp.tile([P, KI, TCH], bf16, tag="xt", name="xt0")
        wq_t = wpool.tile([P, KI, HPC * DH], bf16)
        Q4 = KI // 4
        nc.scalar.dma_start(wq_t[:, :Q4], wqt[:, :Q4])
        nc.sync.dma_start(xt_tiles[0][:, :Q4], xtc[0][:, :Q4])
        for qq in range(1, 4):
            sl = slice(qq * Q4, (qq + 1) * Q4)
            nc.scalar.dma_start(wq_t[:, sl], wqt[:, sl])
            nc.sync.dma_start(xt_tiles[0][:, sl], xtc[0][:, sl])

        # ---- remaining constants / weights, ordered by first-use time.
        # sync ring continues with wk + xt1 halves (needed ~12/21us);
        # scalar ring carries cos/sin (rope @~10us), wv (~18us), wo (last).
        wk_t = wpool.tile([P, KI, HPC * DH], bf16)
        nc.sync.dma_start(wk_t[:, :KI // 2], wkt[:, :KI // 2])
        nc.sync.dma_start(wk_t[:, KI // 2:], wkt[:, KI // 2:])
        xt_tiles[1] = xtp.tile([P, KI, TCH], bf16, tag="xt", name="xt1")
        nc.sync.dma_start(xt_tiles[1][:, :KI // 2], xtc[1][:, :KI // 2])
        nc.sync.dma_start(xt_tiles[1][:, KI // 2:], xtc[1][:, KI // 2:])
        cos2_t = consts.tile([P, S], fp32)
        nc.scalar.dma_start(cos2_t[:, :TCH], cos2[:, :TCH])
        sinp_t = consts.tile([HALF, S], fp32)
        nc.scalar.dma_start(sinp_t[:, :TCH], sinp[:, :TCH])
        wv_t = wpool.tile([P, KI, HPC * DH], bf16)
        nc.scalar.dma_start(wv_t[:, :KI // 2], wvt[:, :KI // 2])
        nc.scalar.dma_start(wv_t[:, KI // 2:], wvt[:, KI // 2:])
        nc.scalar.dma_start(cos2_t[:, TCH:], cos2[:, TCH:])
        nc.scalar.dma_start(sinp_t[:, TCH:], sinp[:, TCH:])
        tril_t = consts.tile([P, P], bf16)
        nc.scalar.dma_start(tril_t[:], tril[:])
        ones_mat = consts.tile([P, P], bf16)
        nc.vector.memset(ones_mat[:], 1.0)
        # xt2/xt3 pre-issued on the scalar ring: nothing queues behind
        # them there, so their WAR waits (xt slot reuse) block nothing.
        for c in (2, 3):
            xt_tiles[c] = xtp.tile([P, KI, TCH], bf16, tag="xt",
                                   name=f"xt{c}")
            nc.scalar.dma_start(xt_tiles[c][:], xtc[c])
        wo_t = wpool.tile([P, HPC, H], bf16)
        nc.scalar.dma_start(wo_t[:], wot[:])

        # ---- per (head, batch) activation stores ----
        qT = [[qkv.tile([P, S], bf16, tag=f"q{h}{b}", name=f"q{h}{b}")
               for b in range(N)] for h in range(HPC)]
        kT = [[qkv.tile([P, S], bf16, tag=f"k{h}{b}", name=f"k{h}{b}")
               for b in range(N)] for h in range(HPC)]
        vS = [qkv.tile([P, SB, HPC * DH], bf16, tag=f"v{b}", name=f"v{b}")
              for b in range(N)]

        def rope_evict(ps, dst, s0):
            # dst[:, s0:s0+TCH] = bf16(RoPE(ps)); ps is [128, TCH] f32 PSUM
            ra = tmp.tile([P, TCH], fp32, tag="ropeA")
            rb = tmp.tile([P, TCH], fp32, tag="ropeB")
            cs = slice(s0, s0 + TCH)
            nc.vector.tensor_mul(ra[:], ps[:], cos2_t[:, cs])
            nc.vector.tensor_mul(rb[:HALF, :], ps[HALF:, :], sinp_t[:, cs])
            nc.vector.tensor_mul(rb[HALF:, :], ps[:HALF, :], sinp_t[:, cs])
            nc.vector.tensor_sub(dst[:HALF, cs], ra[:HALF, :], rb[:HALF, :])
            nc.vector.tensor_add(dst[HALF:, cs], ra[HALF:, :], rb[HALF:, :])

        def emit_attention(b, qc, op_q):
            """Emit one (batch, q-chunk) attention unit; interleave the
            pending output-projection closures (op_q) into PE idle slots.
            Returns the new closure list for this q-chunk's out-proj."""
            q0 = qc * QCH
            nkb = (q0 + QCH) // P       # causal k-block count
            npair = nkb // 2

            def drain(n):
                for _ in range(n):
                    if op_q:
                        op_q.popleft()()

            wt = [None, None]
            sps = [None, None]
            cps = [None, None]
            rbc = [None, None]

            def score_pair(h, kp):
                ps = ps_sc.tile([P, 2, QCH], fp32, tag="sc")
                diag = kp >= npair - 2
                for j in (0, 1):
                    kb = 2 * kp + j
                    w = max(kb * P - q0, 0)
                    nc.tensor.matmul(ps[:, j, w:],
                                     kT[h][b][:, kb * P:(kb + 1) * P],
                                     qT[h][b][:, q0 + w:q0 + QCH],
                                     start=True, stop=True,
                                     skip_group_check=True)
                if not diag:
                    nc.scalar.activation(wt[h][:, 2 * kp:2 * kp + 2, :],
                                         ps[:], EXP, scale=inv_sqrt_dh)
                else:
                    for j in (0, 1):
                        kb = 2 * kp + j
                        w = max(kb * P - q0, 0)
                        nc.scalar.activation(wt[h][:, kb, w:], ps[:, j, w:],
                                             EXP, scale=inv_sqrt_dh)
                        nc.vector.tensor_mul(wt[h][:, kb, w:w + P],
                                             wt[h][:, kb, w:w + P],
                                             tril_t[:])

            def denom_mm(h, kb):
                dd = max(kb * P - q0, 0)
                nc.tensor.matmul(sps[h][:, dd:], ones_mat[:],
                                 wt[h][:, kb, dd:],
                                 start=(kb == 0), stop=(kb == nkb - 1),
                                 skip_group_check=True)

            def ctx_mm(h, kb):
                dd = max(kb * P - q0, 0)
                nc.tensor.matmul(cps[h][:, dd:],
                                 vS[b][:, kb, h * DH:(h + 1) * DH],
                                 wt[h][:, kb, dd:],
                                 start=(kb == 0), stop=(kb == nkb - 1),
                                 skip_group_check=True)

            def recip_tail(h):
                # rbc[h] = exp(-ln(sum)) = 1/sum, broadcast on all rows
                rbs = tmp.tile([P, QCH], fp32, tag="rbs")
                nc.scalar.activation(rbs[:], sps[h][:], LN)
                rbc[h] = tmp.tile([P, QCH], fp32, tag="rbc", name=f"rbc{h}")
                nc.scalar.activation(rbc[h][:], rbs[:], EXP, scale=-1.0)

            ctxT = ctx_pool.tile([P, HPC, QCH], bf16, tag="ctx")

            # Phase A: scores h0 | denom h0 trailing one pair | filler
            wt[0] = wt_pool.tile([P, SB, QCH], bf16, tag="wt", name="wt0")
            sps[0] = ps_dn.tile([P, QCH], fp32, tag="dn", name="sps0")
            for kp in range(npair):
                score_pair(0, kp)
                if kp >= 1:
                    denom_mm(0, 2 * kp - 2)
                    denom_mm(0, 2 * kp - 1)
                drain(2)
            drain(2)
            denom_mm(0, nkb - 2)
            denom_mm(0, nkb - 1)
            recip_tail(0)
            # Phase B: scores h1 | denom h1 trailing one pair
            wt[1] = wt_pool.tile([P, SB, QCH], bf16, tag="wt", name="wt1")
            sps[1] = ps_dn.tile([P, QCH], fp32, tag="dn", name="sps1")
            for kp in range(npair):
                score_pair(1, kp)
                if kp >= 1:
                    denom_mm(1, 2 * kp - 2)
                    denom_mm(1, 2 * kp - 1)
                drain(1)
            drain(1)
            denom_mm(1, nkb - 2)
            denom_mm(1, nkb - 1)
            recip_tail(1)
            # Phase C: context h0 (drain first: covers the cx-bank wait)
            cps[0] = ps_cx.tile([P, QCH], fp32, tag="cx", name="cps0")
            for kp in range(npair):
                drain(1)
                ctx_mm(0, 2 * kp)
                ctx_mm(0, 2 * kp + 1)
            nc.vector.tensor_mul(ctxT[:, 0, :], cps[0][:], rbc[0][:])
            # Phase D: context h1
            cps[1] = ps_cx.tile([P, QCH], fp32, tag="cx", name="cps1")
            for kp in range(npair):
                drain(1)
                ctx_mm(1, 2 * kp)
                ctx_mm(1, 2 * kp + 1)
            nc.vector.tensor_mul(ctxT[:, 1, :], cps[1][:], rbc[1][:])
            drain(len(op_q))

            # stash this q-chunk's output projection as closures
            ot_box = [None]
            new_ops = []
            last = (b == N - 1 and qc == n_qch - 1)

            def make_unit(ts_, hc):
                def unit():
                    if hc == 0:
                        ot_box[0] = outp.tile([P, H], bf16, tag="otile", name="otile")
                    ot = ot_box[0]
                    if last and (ts_ * 4 + hc) % 2:
                        # final drain has no filler: alternate onto the
                        # idle score banks to double the pipeline depth
                        pso = ps_sc.tile([P, 2, 512], fp32, tag="sc",
                                         name="pso_sc")[:, 0, :]
                    else:
                        pso = ps_op.tile([P, 512], fp32, tag="op")
                    for h in range(HPC):
                        nc.tensor.matmul(
                            pso[:], ctxT[:, h, ts_ * P:(ts_ + 1) * P],
                            wo_t[:, h, hc * 512:(hc + 1) * 512],
                            start=(h == 0), stop=(h == HPC - 1),
                            skip_group_check=True)
                    dsl = slice(hc * 512, (hc + 1) * 512)
                    if hc % 2 == 0:
                        nc.scalar.activation(ot[:, dsl], pso[:], COPY)
                    else:
                        nc.vector.tensor_copy(ot[:, dsl], pso[:])
                    if hc == 3:
                        r0 = b * S + q0 + ts_ * P
                        if last and ts_ == 3:
                            nc.sync.dma_start(out[r0:r0 + P, :H // 2],
                                              ot[:, :H // 2])
                            nc.scalar.dma_start(out[r0:r0 + P, H // 2:],
                                                ot[:, H // 2:])
                        else:
                            nc.sync.dma_start(out[r0:r0 + P, :], ot[:])
                return unit

            for ts_ in range(QCH // P):
                for hc in range(H // 512):
                    new_ops.append(make_unit(ts_, hc))
            return deque(new_ops)

        def emit_proj_chunk(c, op_q, first, defer_v=False):
            t0 = c * TCH
            b = t0 // S
            s0 = t0 - b * S
            if c not in xt_tiles:
                xt_tiles[c] = xtp.tile([P, KI, TCH], bf16, tag="xt",
                                       name=f"xt{c}")
                nc.sync.dma_start(xt_tiles[c][:], xtc[c])
            xt_t = xt_tiles[c]

            if first:
                # chunk 0: all Q before K so early mms only need wq
                qk_order = [(wq_t, qT, 0), (wq_t, qT, 1),
                            (wk_t, kT, 0), (wk_t, kT, 1)]
            else:
                qk_order = [(wq_t, qT, 0), (wk_t, kT, 0),
                            (wq_t, qT, 1), (wk_t, kT, 1)]
            for (wsb, dstT, h) in qk_order:
                d0 = h * DH
                ps = ps_op.tile([P, TCH], fp32, tag="op")
                for k in range(KI):
                    nc.tensor.matmul(ps[:], wsb[:, k, d0:d0 + DH],
                                     xt_t[:, k, :],
                                     start=(k == 0), stop=(k == KI - 1))
                rope_evict(ps, dstT[h][b], s0)

            if defer_v:
                # Return V as filler closures (drained into the next
                # attention unit's warmup). They use the idle dn/cx
                # banks so the score pipeline's sc slots stay free.
                def make_v(ts_):
                    def cl():
                        # cx bank: idle until the consuming chunk's
                        # phase C (dn is held by phase A's denominator)
                        psv = ps_cx.tile([P, 512], fp32, tag="cx",
                                         name=f"vd{ts_}")
                        ps = psv[:, :HPC * DH]
                        for k in range(KI):
                            nc.tensor.matmul(
                                ps[:], xt_t[:, k, ts_ * P:(ts_ + 1) * P],
                                wv_t[:, k, :],
                                start=(k == 0), stop=(k == KI - 1))
                        blk = s0 // P + ts_
                        nc.scalar.activation(vS[b][:, blk, :], ps[:], COPY)
                    return cl
                return [make_v(t) for t in range(TCH // P)]

            # V: natural [t, d] layout, two token-subtiles per pair
            for tp in range(TCH // P // 2):
                psv = ps_sc.tile([P, 2, 512], fp32, tag="sc")
                for j in (0, 1):
                    ts_ = 2 * tp + j
                    ps = psv[:, j, :HPC * DH]
                    for k in range(KI):
                        nc.tensor.matmul(
                            ps[:], xt_t[:, k, ts_ * P:(ts_ + 1) * P],
                            wv_t[:, k, :],
                            start=(k == 0), stop=(k == KI - 1))
                blk = s0 // P + 2 * tp
                nc.scalar.activation(vS[b][:, blk:blk + 2, :],
                                     psv[:, :, :HPC * DH], COPY)
            return None

        op_q = deque()
        for _rep in range(repeat):
            # ---- projections b=0 | leftover out-proj of prev rep ----
            for c in range(n_tch // 2 - 1):
                emit_proj_chunk(c, op_q, first=(c == 0 and _rep == 0))
            # chunk 3's V matmuls become warmup filler for attention b0
            vcl = emit_proj_chunk(n_tch // 2 - 1, op_q, first=False,
                                  defer_v=True)
            # ---- attention b=0 (xt for b=1 transfers meanwhile) ----
            for c in (4, 5):
                xt_tiles[c] = xtp.tile([P, KI, TCH], bf16, tag="xt",
                                       name=f"xt{c}")
                nc.sync.dma_start(xt_tiles[c][:], xtc[c])
            op_q.extend(vcl)
            # qc1 first: qc0's thin phases then run filled by qc1's
            # out-proj units instead of draining the pipeline.
            for qc in (1, 0, 2, 3):
                op_q = emit_attention(0, qc, op_q)
            # ---- projections b=1 | out-proj of (b0, qc3) ----
            for c in range(n_tch // 2, n_tch):
                emit_proj_chunk(c, op_q, first=False)
            # ---- attention b=1 ----
            for qc in (1, 0, 2, 3):
                op_q = emit_attention(1, qc, op_q)
            xt_tiles.clear()
        while op_q:
            op_q.popleft()()

    # Steer insert_act_table_loads to the one table set that holds BOTH
    # Exp and Ln ("natural_log_exp_and_others"): blank the exp-only /
    # ln-only sets for the duration of compile so the greedy chooser
    # can't split them across two tables (32 table swaps otherwise).
    # Set ids index the unmodified act_info.json, so the NEFF is
    # unaffected; the cached dict is restored right after.
    from concourse.hw_specs import get_activation_tables
    tabs = get_activation_tables(nc.m.arch)
    saved = {}
    for name, fns in tabs.items():
        if name == "natural_log_exp_and_others":
            continue
        if EXP in fns or LN in fns:
            saved[name] = set(fns)
            fns.clear()
    try:
        nc.compile()
    finally:
        for name, fns in saved.items():
            tabs[name] |= fns
    return nc


@lru_cache(maxsize=2)
def _get_nc(repeat=1):
    return _build_nc(repeat)


def _host_prep(X, position_ids, Wq, Wk, Wv, Wo):
    bf = ml_dtypes.bfloat16
    # xtc[c, p, k, t] = X.T[k*128+p, c*512+t]
    xtb = np.ascontiguousarray(X.reshape(T, H).T).astype(bf)   # [H, T]
    xtc = np.ascontiguousarray(
        xtb.reshape(KI, P, T // TCH, TCH).transpose(2, 1, 0, 3))

    pos = np.asarray(position_ids).astype(np.float64)
    j = np.arange(HALF, dtype=np.float64)
    theta = 1.0 / (10000.0 ** (2.0 * j / DH))
    ang = pos[:, None] * theta[None, :]            # [S, half]
    cosv = np.cos(ang).T.astype(np.float32)        # [half, S]
    sinv = np.sin(ang).T.astype(np.float32)
    cos2 = np.concatenate([cosv, cosv], axis=0)    # [128, S]

    trilm = (np.arange(P)[:, None] <= np.arange(P)[None, :]).astype(bf)

    in_maps = []
    for c in range(N_CORES):
        r0, r1 = c * HPC * DH, (c + 1) * HPC * DH
        in_maps.append({
            "xtc": xtc,
            "wqt": np.ascontiguousarray(
                Wq[r0:r1, :].T.reshape(KI, P, HPC * DH)
                .transpose(1, 0, 2)).astype(bf),
            "wkt": np.ascontiguousarray(
                Wk[r0:r1, :].T.reshape(KI, P, HPC * DH)
                .transpose(1, 0, 2)).astype(bf),
            "wvt": np.ascontiguousarray(
                Wv[r0:r1, :].T.reshape(KI, P, HPC * DH)
                .transpose(1, 0, 2)).astype(bf),
            "wot": np.ascontiguousarray(
                Wo[:, r0:r1].T.reshape(HPC, DH, H)
                .transpose(1, 0, 2)).astype(bf),
            "cos2": cos2, "sinp": sinv, "tril": trilm,
        })
    return in_maps


# ---- cached jitted executable (avoids re-trace/re-compile per call) ----

_EXE_CACHE = {}


def _get_executable(nc):
    key = id(nc)
    if key in _EXE_CACHE:
        return _EXE_CACHE[key]
    import jax
    from jax.sharding import Mesh, PartitionSpec
    try:
        from jax.experimental.shard_map import shard_map
    except ImportError:
        from jax import shard_map
    from concourse.bass2jax import (
        install_neuronx_cc_hook, _bass_exec_p, partition_id_tensor,
    )
    import concourse.mybir as mybir

    install_neuronx_cc_hook()
    partition_name = (nc.partition_id_tensor.name
                      if nc.partition_id_tensor else None)
    in_names, out_names, out_avals, zero_shapes = [], [], [], []
    for alloc in nc.m.functions[0].allocations:
        if not isinstance(alloc, mybir.MemoryLocationSet):
            continue
        name = alloc.memorylocations[0].name
        if alloc.kind == "ExternalInput":
            if name != partition_name:
                in_names.append(name)
        elif alloc.kind == "ExternalOutput":
            shape = tuple(alloc.tensor_shape)
            dtype = mybir.dt.np(alloc.dtype)
            out_avals.append(jax.core.ShapedArray(shape, dtype))
            out_names.append(name)
            zero_shapes.append((shape, dtype))
    n_params = len(in_names)
    in_names_all = in_names + out_names
    if partition_name is not None:
        in_names_all.append(partition_name)

    def _body(*args):
        operands = list(args)
        if partition_name is not None:
            operands.append(partition_id_tensor())
        return tuple(_bass_exec_p.bind(
            *operands, out_avals=tuple(out_avals),
            in_names=tuple(in_names_all), out_names=tuple(out_names),
            lowering_input_output_aliases=(),
            sim_require_finite=True, sim_require_nnan=True, nc=nc))

    devices = jax.devices()[:N_CORES]
    mesh = Mesh(np.asarray(devices), ("core",))
    in_specs = (PartitionSpec("core"),) * (n_params + len(out_names))
    out_specs = (PartitionSpec("core"),) * len(out_names)
    try:
        smapped = shard_map(_body, mesh=mesh, in_specs=in_specs,
                            out_specs=out_specs, check_rep=False)
    except TypeError:
        smapped = shard_map(_body, mesh=mesh, in_specs=in_specs,
                            out_specs=out_specs, check_vma=False)
    sharded = jax.jit(smapped, keep_unused=True)
    exe = (sharded, in_names, out_names, out_avals, zero_shapes)
    _EXE_CACHE[key] = exe
    return exe


def _run_cached(nc, in_maps):
    sharded, in_names, out_names, out_avals, zero_shapes = _get_executable(nc)
    per_core = [[np.asarray(m[name]) for name in in_names] for m in in_maps]
    concat_in = [
        np.concatenate([per_core[c][i] for c in range(N_CORES)], axis=0)
        for i in range(len(in_names))]
    concat_zeros = [
        np.zeros((N_CORES * shape[0], *shape[1:]), dtype)
        for shape, dtype in zero_shapes]
    out_arrs = sharded(*concat_in, *concat_zeros)
    return [
        {name: np.asarray(out_arrs[i]).reshape(N_CORES,
                                               *out_avals[i].shape)[c]
         for i, name in enumerate(out_names)}
        for c in range(N_CORES)]


def run_once(in_maps, repeat=1):
    nc = _get_nc(repeat)
    return _run_cached(nc, in_maps)


def kernel(X, position_ids, mask, Wq, Wk, Wv, Wo, bo, _trace=False):
    X = np.asarray(X, dtype=np.float32)
    in_maps = _host_prep(X, position_ids,
                         np.asarray(Wq, dtype=np.float32),
                         np.asarray(Wk, dtype=np.float32),
                         np.asarray(Wv, dtype=np.float32),
                         np.asarray(Wo, dtype=np.float32))

    nc = _get_nc()
    if _trace:
        from concourse.bass_utils import run_bass_kernel_spmd
        res = run_bass_kernel_spmd(nc, in_maps, list(range(N_CORES)),
                                   trace=True)
        results = res.results
    else:
        res = None
        results = _run_cached(nc, in_maps)
    acc = np.zeros((T, H), dtype=np.float32)
    for c in range(N_CORES):
        acc += results[c]["out"].astype(np.float32)
    acc += np.asarray(bo, dtype=np.float32)[None, :]
    out = acc.reshape(N, S, H)
    if _trace:
        return out, res
    return out
